# revision 2
# baseline (speedup 1.0000x reference)
"""MiniDeepSeekV3Gate (noaux-topk MoE routing) Trainium2 Bass kernel.

Problem: T=16384 tokens, H=2048 hidden, E=256 experts, 8 groups of 32,
top-2-per-group sums -> top-4 groups -> top-8 experts -> normalized
sigmoid gate weights (scaled 2.5) + int32 expert indices.

Sharding: pure data parallel over tokens. Each of the 8 NeuronCores gets
2048 tokens and a replicated copy of the gate weight (host-prepped as a
transposed bf16 hi/lo pair) + bias. No cross-core communication.

Per-core dataflow (3-pass bf16 split matmul):
  - x is split into bf16 hi = rn(x), lo = rn(x - hi); W likewise into
    Wh + Wl on the host. scores = Wh.xh + Wl.xh + Wh.xl in fp32 PSUM
    (dropped Wl.xl term is ~2^-18 relative: far below the top-k
    tie-break sensitivity that rules out tf32/bf16 single-pass).
    bf16 matmuls stream at 1 cyc/row vs 4 for fp32, so 3 passes cost
    75% of one fp32 pass.
  - matmul orientation: x^T chunks are the STATIONARY operand, W^T
    [128h, 256e] the moving one -> PSUM scores come out token-major
    [128t, 256e], so no transpose is needed between scores and routing.
  - the hi/lo transposes to hidden-major are split between the DMA xbar
    transpose unit (16-bit SBUF->SBUF, 14ns per 16x128 tile) and the PE
    (fp32 transpose of x then split during PSUM evacuation): DMA copies
    serialize on a single 360GB/s resource that also carries the 16MB
    x load, while PE carries the 196k-cycle matmul stream -> balance.
  - routing chain per 128-token block on VectorE: per-group Max8 ->
    group top-2 sums -> top-4 group threshold mask -> masked
    Max8/MaxIndex over 256 -> normalize.  lo-subtracts for xbar blocks
    run on GpSimd to keep VectorE headroom.
"""

import numpy as np

import concourse.bass as bass
import concourse.tile as tile
from concourse import bacc, mybir
from concourse.bass_utils import run_bass_kernel_spmd
from concourse.masks import make_identity

F32 = mybir.dt.float32
BF16 = mybir.dt.bfloat16
I32 = mybir.dt.int32
U32 = mybir.dt.uint32
SIG = mybir.ActivationFunctionType.Sigmoid
ALU = mybir.AluOpType

N_CORES = 8
T_FULL = 16384
T_CORE = T_FULL // N_CORES  # 2048
HID = 2048
NE = 256
NG = 8
EPG = 32
TOPK = 8
ROUTE_SCALE = 2.5
NK = HID // 128          # 16 contraction chunks
NB = T_CORE // 128       # 16 token blocks per core
BIG = 1.0e30

# token blocks whose hi/lo transposes run on the PE (fp32 transpose +
# split-during-evacuation); the rest go through the DMA xbar transpose.
PE_BLOCKS = frozenset((0, 3, 6, 9, 12))


def build_nc():
    nc = bacc.Bacc("TRN2", target_bir_lowering=False, debug=False,
                   num_devices=N_CORES)
    x = nc.dram_tensor("hidden_states", [T_CORE, HID], F32,
                       kind="ExternalInput").ap()
    wht = nc.dram_tensor("wht", [128, NK, NE], BF16, kind="ExternalInput").ap()
    wlt = nc.dram_tensor("wlt", [128, NK, NE], BF16, kind="ExternalInput").ap()
    b = nc.dram_tensor("bias", [NE], F32, kind="ExternalInput").ap()
    out_w = nc.dram_tensor("weights_out", [T_CORE, TOPK], F32,
                           kind="ExternalOutput").ap()
    out_i = nc.dram_tensor("indices_out", [T_CORE, TOPK], I32,
                           kind="ExternalOutput").ap()

    with tile.TileContext(nc) as tc:
        build_tile_kernel(tc, x, wht, wlt, b, out_w, out_i)
    nc.compile()
    return nc


def build_tile_kernel(tc, x, wht, wlt, b, out_w, out_i):
    nc = tc.nc
    from contextlib import ExitStack
    ctx = ExitStack()
    with ctx:
        consts = ctx.enter_context(tc.tile_pool(name="consts", bufs=1))
        xn_pool = ctx.enter_context(tc.tile_pool(name="xn", bufs=4))
        hl_pool = ctx.enter_context(tc.tile_pool(name="hl", bufs=4))
        xt_pool = ctx.enter_context(tc.tile_pool(name="xt", bufs=3))
        st_pool = ctx.enter_context(tc.tile_pool(name="st", bufs=3))
        rt_pool = ctx.enter_context(tc.tile_pool(name="rt", bufs=3))
        ps_mm = ctx.enter_context(tc.tile_pool(name="ps_mm", bufs=3,
                                               space="PSUM"))
        ps_tp = ctx.enter_context(tc.tile_pool(name="ps_tp", bufs=3,
                                               space="PSUM"))

        # ---- constants / weights ----
        ident = consts.tile([128, 128], F32)
        make_identity(nc, ident[:])
        wh = consts.tile([128, NK, NE], BF16)
        wl = consts.tile([128, NK, NE], BF16)
        bias_bc = consts.tile([128, NE], F32)
        # staging for the outputs (written per block, flushed once)
        wo = consts.tile([128, NB, TOPK], F32)
        io = consts.tile([128, NB, TOPK], U32)

        state = {}

        def emit_load(tb):
            xn = xn_pool.tile([128, HID], F32, name=f"xn_{tb}", tag="xn")
            nc.sync.dma_start(xn[:], x[tb * 128:(tb + 1) * 128, :])
            state[("xn", tb)] = xn
            if tb == 1:
                # weights/bias after the first x tile so the PE pipeline
                # primes as early as possible
                nc.sync.dma_start(wh[:], wht)
                nc.sync.dma_start(wl[:], wlt)
                nc.sync.dma_start(bias_bc[:],
                                  b.unsqueeze(0).partition_broadcast(128))

        def emit_split(tb):
            # token-major hi/lo split for xbar-path blocks
            if tb in PE_BLOCKS:
                return
            xn = state[("xn", tb)]
            hi = hl_pool.tile([128, HID], BF16, name=f"hi_{tb}", tag="hi")
            lo = hl_pool.tile([128, HID], BF16, name=f"lo_{tb}", tag="lo")
            nc.scalar.copy(hi[:], xn[:])
            nc.gpsimd.tensor_tensor(out=lo[:], in0=xn[:], in1=hi[:],
                                    op=ALU.subtract)
            state[("hi", tb)] = hi
            state[("lo", tb)] = lo

        def emit_transpose(tb):
            xh = xt_pool.tile([128, NK, 128], BF16, name=f"xh_{tb}", tag="xh")
            xl = xt_pool.tile([128, NK, 128], BF16, name=f"xl_{tb}", tag="xl")
            if tb in PE_BLOCKS:
                # PE path: fp32 transpose of x, split during evacuation
                xn = state[("xn", tb)]
                for kg in range(NK // 4):
                    px = ps_tp.tile([128, 4, 128], F32, name=f"px_{tb}_{kg}",
                                    tag="ps_tp")
                    for j in range(4):
                        k = kg * 4 + j
                        nc.tensor.transpose(px[:, j, :],
                                            xn[:, k * 128:(k + 1) * 128],
                                            ident[:])
                    sl = slice(kg * 4, kg * 4 + 4)
                    nc.scalar.copy(xh[:, sl, :], px[:])
                    nc.vector.tensor_tensor(out=xl[:, sl, :], in0=px[:],
                                            in1=xh[:, sl, :],
                                            op=ALU.subtract)
            else:
                nc.sync.dma_start(xh[:], state.pop(("hi", tb))[:],
                                  transpose=True)
                nc.sync.dma_start(xl[:], state.pop(("lo", tb))[:],
                                  transpose=True)
            state[("xh", tb)] = xh
            state[("xl", tb)] = xl

        def emit_mm(tb):
            xh = state.pop(("xh", tb))
            xl = state.pop(("xl", tb))
            ps = ps_mm.tile([128, NE], F32, name=f"ps_{tb}", tag="ps_mm")
            n = 3 * NK
            i = 0
            for k in range(NK):
                nc.tensor.matmul(ps[:], xh[:, k, :], wh[:, k, :],
                                 start=(i == 0), stop=(i == n - 1))
                i += 1
                nc.tensor.matmul(ps[:], xh[:, k, :], wl[:, k, :],
                                 start=False, stop=(i == n - 1))
                i += 1
                nc.tensor.matmul(ps[:], xl[:, k, :], wh[:, k, :],
                                 start=False, stop=(i == n - 1))
                i += 1
            st = st_pool.tile([128, NE], F32, name=f"st_{tb}", tag="st")
            nc.scalar.activation(st[:], ps[:], SIG)
            state[("st", tb)] = st

        def emit_route(tb):
            st = state.pop(("st", tb))
            ssel = rt_pool.tile([128, NE], F32, name=f"ssel_{tb}", tag="ssel")
            nc.vector.tensor_tensor(out=ssel[:], in0=st[:], in1=bias_bc[:],
                                    op=ALU.add)
            gtop = rt_pool.tile([128, NG, 8], F32, name=f"gtop_{tb}",
                                tag="gtop")
            for g in range(NG):
                nc.vector.max(gtop[:, g, :], ssel[:, g * EPG:(g + 1) * EPG])
            g2 = rt_pool.tile([128, NG], F32, name=f"g2_{tb}", tag="g2")
            nc.vector.tensor_tensor(out=g2[:], in0=gtop[:, :, 0],
                                    in1=gtop[:, :, 1], op=ALU.add)
            gs8 = rt_pool.tile([128, NG], F32, name=f"gs8_{tb}", tag="gs8")
            nc.vector.max(gs8[:], g2[:])
            # additive group mask: selected -> 0, unselected -> -BIG
            maskg = rt_pool.tile([128, NG], F32, name=f"mg_{tb}", tag="mg")
            nc.vector.tensor_scalar(out=maskg[:], in0=g2[:],
                                    scalar1=gs8[:, 3:4], scalar2=BIG,
                                    op0=ALU.is_ge, op1=ALU.mult)
            masked = rt_pool.tile([128, NE], F32, name=f"msk_{tb}", tag="msk")
            nc.vector.scalar_tensor_tensor(
                out=masked[:].rearrange("p (g e) -> p g e", g=NG),
                in0=maskg[:].unsqueeze(2).broadcast_to((128, NG, EPG)),
                scalar=BIG,
                in1=ssel[:].rearrange("p (g e) -> p g e", g=NG),
                op0=ALU.subtract, op1=ALU.add)
            top8v = rt_pool.tile([128, TOPK], F32, name=f"t8_{tb}", tag="t8")
            nc.vector.max(top8v[:], masked[:])
            nc.vector.max_index(io[:, tb, :], top8v[:], masked[:])
            ssum = rt_pool.tile([128, 1], F32, name=f"ssum_{tb}", tag="ssum")
            nc.vector.reduce_sum(out=ssum[:], in_=top8v[:],
                                 axis=mybir.AxisListType.X)
            seps = rt_pool.tile([128, 1], F32, name=f"seps_{tb}", tag="seps")
            nc.vector.tensor_scalar_add(seps[:], ssum[:], 1e-6)
            rinv = rt_pool.tile([128, 1], F32, name=f"rinv_{tb}", tag="rinv")
            nc.vector.reciprocal(rinv[:], seps[:])
            nc.vector.tensor_scalar(out=wo[:, tb, :], in0=top8v[:],
                                    scalar1=rinv[:], scalar2=ROUTE_SCALE,
                                    op0=ALU.mult, op1=ALU.mult)

        # ---- software pipeline over token blocks ----
        for i in range(NB + 4):
            if i < NB:
                emit_load(i)
            if 0 <= i - 1 < NB:
                emit_split(i - 1)
            if 0 <= i - 2 < NB:
                emit_transpose(i - 2)
            if 0 <= i - 3 < NB:
                emit_mm(i - 3)
            if 0 <= i - 4 < NB:
                emit_route(i - 4)

        nc.sync.dma_start(
            out_w.rearrange("(tb p) k -> p tb k", tb=NB), wo[:])
        nc.sync.dma_start(
            out_i.rearrange("(tb p) k -> p tb k", tb=NB), io[:].bitcast(I32))


_NC_CACHE = None


def _get_nc():
    global _NC_CACHE
    if _NC_CACHE is None:
        _NC_CACHE = build_nc()
    return _NC_CACHE


def _split_weight(weight):
    """Host weight prep: W [E, H] fp32 -> transposed bf16 hi/lo pair,
    laid out [128, NK, NE] with [p, k, e] = W[e, k*128 + p]."""
    import ml_dtypes
    wt = np.ascontiguousarray(weight.T.astype(np.float32))       # [H, E]
    wh = wt.astype(ml_dtypes.bfloat16)
    wl = (wt - wh.astype(np.float32)).astype(ml_dtypes.bfloat16)
    wh = np.ascontiguousarray(wh.reshape(NK, 128, NE).transpose(1, 0, 2))
    wl = np.ascontiguousarray(wl.reshape(NK, 128, NE).transpose(1, 0, 2))
    return wh, wl


def kernel(hidden_states: np.ndarray, weight: np.ndarray, bias: np.ndarray):
    hidden_states = np.ascontiguousarray(hidden_states, dtype=np.float32)
    weight = np.ascontiguousarray(weight, dtype=np.float32)
    bias = np.ascontiguousarray(bias, dtype=np.float32)
    wh, wl = _split_weight(weight)
    nc = _get_nc()
    in_maps = [
        {
            "hidden_states": hidden_states[c * T_CORE:(c + 1) * T_CORE],
            "wht": wh,
            "wlt": wl,
            "bias": bias,
        }
        for c in range(N_CORES)
    ]
    res = run_bass_kernel_spmd(nc, in_maps, list(range(N_CORES))).results
    weights = np.concatenate([r["weights_out"] for r in res], axis=0)
    indices = np.concatenate([r["indices_out"] for r in res], axis=0)
    return weights.astype(np.float32), indices.astype(np.int32)


# revision 6
# speedup vs baseline: 1.0727x; 1.0727x over previous
"""MiniDeepSeekV3Gate (noaux-topk MoE routing) Trainium2 Bass kernel.

Problem: T=16384 tokens, H=2048 hidden, E=256 experts, 8 groups of 32,
top-2-per-group sums -> top-4 groups -> top-8 experts -> normalized
sigmoid gate weights (scaled 2.5) + int32 expert indices.

Sharding: pure data parallel over tokens. Each of the 8 NeuronCores gets
2048 tokens and a replicated copy of the gate weight (host-prepped as a
transposed bf16 hi/lo pair) + bias. No cross-core communication.

Per-core dataflow (3-pass bf16 split matmul):
  - x is split into bf16 hi = rn(x), lo = rn(x - hi); W likewise into
    Wh + Wl on the host. scores = Wh.xh + Wl.xh + Wh.xl in fp32 PSUM
    (dropped Wl.xl term is ~2^-18 relative: far below the top-k
    tie-break sensitivity that rules out tf32/bf16 single-pass).
    bf16 matmuls stream at 1 cyc/row vs 4 for fp32, so 3 passes cost
    75% of one fp32 pass.
  - matmul orientation: x^T chunks are the STATIONARY operand, W^T
    [128h, 256e] the moving one -> PSUM scores come out token-major
    [128t, 256e], so no transpose is needed between scores and routing.
  - the hi/lo transposes to hidden-major are split between the DMA xbar
    transpose unit (16-bit SBUF->SBUF, 14ns per 16x128 tile) and the PE
    (fp32 transpose of x then split during PSUM evacuation): DMA copies
    serialize on a single 360GB/s resource that also carries the 16MB
    x load, while PE carries the 196k-cycle matmul stream -> balance.
  - routing chain per 128-token block on VectorE: per-group Max8 ->
    group top-2 sums -> top-4 group threshold mask -> masked
    Max8/MaxIndex over 256 -> normalize.  lo-subtracts for xbar blocks
    run on GpSimd to keep VectorE headroom.
"""

import numpy as np

import concourse.bass as bass
import concourse.tile as tile
from concourse import bacc, mybir
from concourse.bass_utils import run_bass_kernel_spmd
from concourse.masks import make_identity

F32 = mybir.dt.float32
BF16 = mybir.dt.bfloat16
I32 = mybir.dt.int32
U32 = mybir.dt.uint32
SIG = mybir.ActivationFunctionType.Sigmoid
ALU = mybir.AluOpType

N_CORES = 8
T_FULL = 16384
T_CORE = T_FULL // N_CORES  # 2048
HID = 2048
NE = 256
NG = 8
EPG = 32
TOPK = 8
ROUTE_SCALE = 2.5
NK = HID // 128          # 16 contraction chunks
NB = T_CORE // 128       # 16 token blocks per core
BIG = 1.0e30

# token blocks whose hi/lo transposes run on the PE (fp32 transpose +
# split-during-evacuation); the rest go through the DMA xbar transpose.
PE_BLOCKS = frozenset((0, 3, 6, 9, 12))


def build_nc():
    nc = bacc.Bacc("TRN2", target_bir_lowering=False, debug=False,
                   num_devices=N_CORES)
    x = nc.dram_tensor("hidden_states", [T_CORE, HID], F32,
                       kind="ExternalInput").ap()
    wht = nc.dram_tensor("wht", [128, NK, NE], BF16, kind="ExternalInput").ap()
    wlt = nc.dram_tensor("wlt", [128, NK, NE], BF16, kind="ExternalInput").ap()
    b = nc.dram_tensor("bias", [NE], F32, kind="ExternalInput").ap()
    out_w = nc.dram_tensor("weights_out", [T_CORE, TOPK], F32,
                           kind="ExternalOutput").ap()
    out_i = nc.dram_tensor("indices_out", [T_CORE, TOPK], I32,
                           kind="ExternalOutput").ap()

    with tile.TileContext(nc) as tc:
        build_tile_kernel(tc, x, wht, wlt, b, out_w, out_i)
    nc.compile()
    return nc


def build_tile_kernel(tc, x, wht, wlt, b, out_w, out_i):
    nc = tc.nc
    from contextlib import ExitStack
    ctx = ExitStack()
    with ctx:
        consts = ctx.enter_context(tc.tile_pool(name="consts", bufs=1))
        xn_pool = ctx.enter_context(tc.tile_pool(name="xn", bufs=4))
        hl_pool = ctx.enter_context(tc.tile_pool(name="hl", bufs=4))
        xt_pool = ctx.enter_context(tc.tile_pool(name="xt", bufs=4))
        st_pool = ctx.enter_context(tc.tile_pool(name="st", bufs=3))
        rt_pool = ctx.enter_context(tc.tile_pool(name="rt", bufs=3))
        ps_mm = ctx.enter_context(tc.tile_pool(name="ps_mm", bufs=3,
                                               space="PSUM"))
        ps_tp = ctx.enter_context(tc.tile_pool(name="ps_tp", bufs=3,
                                               space="PSUM"))

        # ---- constants / weights ----
        ident = consts.tile([128, 128], F32)
        make_identity(nc, ident[:])
        wh = consts.tile([128, NK, NE], BF16)
        wl = consts.tile([128, NK, NE], BF16)
        bias_bc = consts.tile([128, NE], F32)
        # staging for the outputs (written per block, flushed once)
        wo = consts.tile([128, NB, TOPK], F32)
        io = consts.tile([128, NB, TOPK], U32)

        state = {}

        def emit_load(tb):
            xn = xn_pool.tile([128, HID], F32, name=f"xn_{tb}", tag="xn")
            nc.sync.dma_start(xn[:], x[tb * 128:(tb + 1) * 128, :])
            state[("xn", tb)] = xn
            if tb == 0:
                # Wh right after the first x tile (first 32 matmuls per
                # block touch only Wh), Wl a block later, bias after that
                nc.sync.dma_start(wh[:], wht)
            elif tb == 1:
                nc.sync.dma_start(wl[:], wlt)
            elif tb == 2:
                nc.sync.dma_start(bias_bc[:],
                                  b.unsqueeze(0).partition_broadcast(128))

        def emit_split(tb):
            # token-major hi/lo split for xbar-path blocks
            if tb in PE_BLOCKS:
                return
            xn = state[("xn", tb)]
            hi = hl_pool.tile([128, HID], BF16, name=f"hi_{tb}", tag="hi")
            lo = hl_pool.tile([128, HID], BF16, name=f"lo_{tb}", tag="lo")
            nc.scalar.copy(hi[:], xn[:])
            nc.gpsimd.tensor_tensor(out=lo[:], in0=xn[:], in1=hi[:],
                                    op=ALU.subtract)
            state[("hi", tb)] = hi
            state[("lo", tb)] = lo

        def emit_transpose(tb):
            xh = xt_pool.tile([128, NK, 128], BF16, name=f"xh_{tb}", tag="xh")
            xl = xt_pool.tile([128, NK, 128], BF16, name=f"xl_{tb}", tag="xl")
            if tb in PE_BLOCKS:
                # PE path: fp32 transpose of x, split during evacuation
                xn = state[("xn", tb)]
                for kg in range(NK // 4):
                    px = ps_tp.tile([128, 4, 128], F32, name=f"px_{tb}_{kg}",
                                    tag="ps_tp")
                    for j in range(4):
                        k = kg * 4 + j
                        nc.tensor.transpose(px[:, j, :],
                                            xn[:, k * 128:(k + 1) * 128],
                                            ident[:])
                    sl = slice(kg * 4, kg * 4 + 4)
                    nc.scalar.copy(xh[:, sl, :], px[:])
                    nc.vector.tensor_tensor(out=xl[:, sl, :], in0=px[:],
                                            in1=xh[:, sl, :],
                                            op=ALU.subtract)
            else:
                nc.sync.dma_start(xh[:], state.pop(("hi", tb))[:],
                                  transpose=True)
                nc.sync.dma_start(xl[:], state.pop(("lo", tb))[:],
                                  transpose=True)
            state[("xh", tb)] = xh
            state[("xl", tb)] = xl

        def emit_mm(tb):
            xh = state.pop(("xh", tb))
            xl = state.pop(("xl", tb))
            ps = ps_mm.tile([128, NE], F32, name=f"ps_{tb}", tag="ps_mm")
            # xh passes first: for xbar blocks the xl tiles (second xbar
            # DMA) arrive while the first 32 matmuls already stream
            n = 3 * NK
            i = 0
            for xs, ws in ((xh, wh), (xh, wl), (xl, wh)):
                for k in range(NK):
                    nc.tensor.matmul(ps[:], xs[:, k, :], ws[:, k, :],
                                     start=(i == 0), stop=(i == n - 1))
                    i += 1
            st = st_pool.tile([128, NE], F32, name=f"st_{tb}", tag="st")
            nc.scalar.activation(st[:], ps[:], SIG)
            state[("st", tb)] = st

        def emit_route(tb):
            st = state.pop(("st", tb))
            ssel = rt_pool.tile([128, NE], F32, name=f"ssel_{tb}", tag="ssel")
            nc.vector.tensor_tensor(out=ssel[:], in0=st[:], in1=bias_bc[:],
                                    op=ALU.add)
            gtop = rt_pool.tile([128, NG, 8], F32, name=f"gtop_{tb}",
                                tag="gtop")
            for g in range(NG):
                nc.vector.max(gtop[:, g, :], ssel[:, g * EPG:(g + 1) * EPG])
            g2 = rt_pool.tile([128, NG], F32, name=f"g2_{tb}", tag="g2")
            nc.vector.tensor_tensor(out=g2[:], in0=gtop[:, :, 0],
                                    in1=gtop[:, :, 1], op=ALU.add)
            gs8 = rt_pool.tile([128, NG], F32, name=f"gs8_{tb}", tag="gs8")
            nc.vector.max(gs8[:], g2[:])
            # additive group mask: selected -> 0, unselected -> -BIG
            maskg = rt_pool.tile([128, NG], F32, name=f"mg_{tb}", tag="mg")
            nc.vector.tensor_scalar(out=maskg[:], in0=g2[:],
                                    scalar1=gs8[:, 3:4], scalar2=BIG,
                                    op0=ALU.is_ge, op1=ALU.mult)
            masked = rt_pool.tile([128, NE], F32, name=f"msk_{tb}", tag="msk")
            nc.vector.scalar_tensor_tensor(
                out=masked[:].rearrange("p (g e) -> p g e", g=NG),
                in0=maskg[:].unsqueeze(2).broadcast_to((128, NG, EPG)),
                scalar=BIG,
                in1=ssel[:].rearrange("p (g e) -> p g e", g=NG),
                op0=ALU.subtract, op1=ALU.add)
            top8v = rt_pool.tile([128, TOPK], F32, name=f"t8_{tb}", tag="t8")
            nc.vector.max(top8v[:], masked[:])
            nc.vector.max_index(io[:, tb, :], top8v[:], masked[:])
            ssum = rt_pool.tile([128, 1], F32, name=f"ssum_{tb}", tag="ssum")
            nc.vector.reduce_sum(out=ssum[:], in_=top8v[:],
                                 axis=mybir.AxisListType.X)
            seps = rt_pool.tile([128, 1], F32, name=f"seps_{tb}", tag="seps")
            nc.vector.tensor_scalar_add(seps[:], ssum[:], 1e-6)
            rinv = rt_pool.tile([128, 1], F32, name=f"rinv_{tb}", tag="rinv")
            nc.vector.reciprocal(rinv[:], seps[:])
            nc.vector.tensor_scalar(out=wo[:, tb, :], in0=top8v[:],
                                    scalar1=rinv[:], scalar2=ROUTE_SCALE,
                                    op0=ALU.mult, op1=ALU.mult)

        # ---- software pipeline over token blocks ----
        # xbar transposes go on the SP queue BEFORE the next x load: they
        # gate the PE while the load only feeds two stages later.
        ow = out_w.rearrange("(tb p) k -> p tb k", tb=NB)
        oi = out_i.rearrange("(tb p) k -> p tb k", tb=NB)

        def emit_flush(lo_b, hi_b):
            nc.sync.dma_start(ow[:, lo_b:hi_b, :], wo[:, lo_b:hi_b, :])
            nc.sync.dma_start(oi[:, lo_b:hi_b, :],
                              io[:, lo_b:hi_b, :].bitcast(I32))

        for i in range(NB + 5):
            if 0 <= i - 2 < NB:
                emit_transpose(i - 2)
            if i < NB:
                emit_load(i)
            if 0 <= i - 1 < NB:
                emit_split(i - 1)
            if 0 <= i - 4 < NB:
                emit_mm(i - 4)
            if 0 <= i - 5 < NB:
                emit_route(i - 5)
            if i - 5 == 10:
                emit_flush(0, 11)
        emit_flush(11, NB)


_NC_CACHE = None


def _get_nc():
    global _NC_CACHE
    if _NC_CACHE is None:
        _NC_CACHE = build_nc()
    return _NC_CACHE


def _split_weight(weight):
    """Host weight prep: W [E, H] fp32 -> transposed bf16 hi/lo pair,
    laid out [128, NK, NE] with [p, k, e] = W[e, k*128 + p]."""
    import ml_dtypes
    wt = np.ascontiguousarray(weight.T.astype(np.float32))       # [H, E]
    wh = wt.astype(ml_dtypes.bfloat16)
    wl = (wt - wh.astype(np.float32)).astype(ml_dtypes.bfloat16)
    wh = np.ascontiguousarray(wh.reshape(NK, 128, NE).transpose(1, 0, 2))
    wl = np.ascontiguousarray(wl.reshape(NK, 128, NE).transpose(1, 0, 2))
    return wh, wl


def kernel(hidden_states: np.ndarray, weight: np.ndarray, bias: np.ndarray):
    hidden_states = np.ascontiguousarray(hidden_states, dtype=np.float32)
    weight = np.ascontiguousarray(weight, dtype=np.float32)
    bias = np.ascontiguousarray(bias, dtype=np.float32)
    wh, wl = _split_weight(weight)
    nc = _get_nc()
    in_maps = [
        {
            "hidden_states": hidden_states[c * T_CORE:(c + 1) * T_CORE],
            "wht": wh,
            "wlt": wl,
            "bias": bias,
        }
        for c in range(N_CORES)
    ]
    res = run_bass_kernel_spmd(nc, in_maps, list(range(N_CORES))).results
    weights = np.concatenate([r["weights_out"] for r in res], axis=0)
    indices = np.concatenate([r["indices_out"] for r in res], axis=0)
    return weights.astype(np.float32), indices.astype(np.int32)


# revision 7
# speedup vs baseline: 1.0740x; 1.0012x over previous
"""MiniDeepSeekV3Gate (noaux-topk MoE routing) Trainium2 Bass kernel.

Problem: T=16384 tokens, H=2048 hidden, E=256 experts, 8 groups of 32,
top-2-per-group sums -> top-4 groups -> top-8 experts -> normalized
sigmoid gate weights (scaled 2.5) + int32 expert indices.

Sharding: pure data parallel over tokens. Each of the 8 NeuronCores gets
2048 tokens and a replicated copy of the gate weight (host-prepped as a
transposed bf16 hi/lo pair) + bias. No cross-core communication.

Per-core dataflow (3-pass bf16 split matmul):
  - x is split into bf16 hi = rn(x), lo = rn(x - hi); W likewise into
    Wh + Wl on the host. scores = Wh.xh + Wl.xh + Wh.xl in fp32 PSUM
    (dropped Wl.xl term is ~2^-18 relative: far below the top-k
    tie-break sensitivity that rules out tf32/bf16 single-pass).
    bf16 matmuls stream at 1 cyc/row vs 4 for fp32, so 3 passes cost
    75% of one fp32 pass.
  - matmul orientation: x^T chunks are the STATIONARY operand, W^T
    [128h, 256e] the moving one -> PSUM scores come out token-major
    [128t, 256e], so no transpose is needed between scores and routing.
  - the hi/lo transposes to hidden-major are split between the DMA xbar
    transpose unit (16-bit SBUF->SBUF, 14ns per 16x128 tile) and the PE
    (fp32 transpose of x then split during PSUM evacuation): DMA copies
    serialize on a single 360GB/s resource that also carries the 16MB
    x load, while PE carries the 196k-cycle matmul stream -> balance.
  - routing chain per 128-token block on VectorE: per-group Max8 ->
    group top-2 sums -> top-4 group threshold mask -> masked
    Max8/MaxIndex over 256 -> normalize.  lo-subtracts for xbar blocks
    run on GpSimd to keep VectorE headroom.
"""

import numpy as np

import concourse.bass as bass
import concourse.tile as tile
from concourse import bacc, mybir
from concourse.bass_utils import run_bass_kernel_spmd
from concourse.masks import make_identity

F32 = mybir.dt.float32
BF16 = mybir.dt.bfloat16
I32 = mybir.dt.int32
U32 = mybir.dt.uint32
SIG = mybir.ActivationFunctionType.Sigmoid
ALU = mybir.AluOpType

N_CORES = 8
T_FULL = 16384
T_CORE = T_FULL // N_CORES  # 2048
HID = 2048
NE = 256
NG = 8
EPG = 32
TOPK = 8
ROUTE_SCALE = 2.5
NK = HID // 128          # 16 contraction chunks
NB = T_CORE // 128       # 16 token blocks per core
BIG = 1.0e30

# token blocks whose hi/lo transposes run on the PE (fp32 transpose +
# split-during-evacuation); the rest go through the DMA xbar transpose.
PE_BLOCKS = frozenset((0, 3, 6, 9, 12))


def build_nc():
    nc = bacc.Bacc("TRN2", target_bir_lowering=False, debug=False,
                   num_devices=N_CORES)
    x = nc.dram_tensor("hidden_states", [T_CORE, HID], F32,
                       kind="ExternalInput").ap()
    wht = nc.dram_tensor("wht", [128, NK, NE], BF16, kind="ExternalInput").ap()
    wlt = nc.dram_tensor("wlt", [128, NK, NE], BF16, kind="ExternalInput").ap()
    b = nc.dram_tensor("bias", [NE], F32, kind="ExternalInput").ap()
    out_w = nc.dram_tensor("weights_out", [T_CORE, TOPK], F32,
                           kind="ExternalOutput").ap()
    out_i = nc.dram_tensor("indices_out", [T_CORE, TOPK], I32,
                           kind="ExternalOutput").ap()

    with tile.TileContext(nc) as tc:
        build_tile_kernel(tc, x, wht, wlt, b, out_w, out_i)
    nc.compile()
    return nc


def build_tile_kernel(tc, x, wht, wlt, b, out_w, out_i):
    nc = tc.nc
    from contextlib import ExitStack
    ctx = ExitStack()
    with ctx:
        consts = ctx.enter_context(tc.tile_pool(name="consts", bufs=1))
        xn_pool = ctx.enter_context(tc.tile_pool(name="xn", bufs=4))
        hl_pool = ctx.enter_context(tc.tile_pool(name="hl", bufs=4))
        xt_pool = ctx.enter_context(tc.tile_pool(name="xt", bufs=5))
        st_pool = ctx.enter_context(tc.tile_pool(name="st", bufs=3))
        rt_pool = ctx.enter_context(tc.tile_pool(name="rt", bufs=3))
        ps_mm = ctx.enter_context(tc.tile_pool(name="ps_mm", bufs=3,
                                               space="PSUM"))
        ps_tp = ctx.enter_context(tc.tile_pool(name="ps_tp", bufs=4,
                                               space="PSUM"))

        # ---- constants / weights ----
        ident = consts.tile([128, 128], F32)
        make_identity(nc, ident[:])
        wh = consts.tile([128, NK, NE], BF16)
        wl = consts.tile([128, NK, NE], BF16)
        bias_bc = consts.tile([128, NE], F32)
        # staging for the outputs (written per block, flushed once)
        wo = consts.tile([128, NB, TOPK], F32)
        io = consts.tile([128, NB, TOPK], U32)

        state = {}

        def emit_load(tb):
            xn = xn_pool.tile([128, HID], F32, name=f"xn_{tb}", tag="xn")
            if tb == 0:
                # chunked first load: the PE can start transposing block 0
                # ~2us earlier than a monolithic 1MB DMA allows
                for q in range(4):
                    nc.sync.dma_start(xn[:, q * 512:(q + 1) * 512],
                                      x[0:128, q * 512:(q + 1) * 512])
            else:
                nc.sync.dma_start(xn[:], x[tb * 128:(tb + 1) * 128, :])
            state[("xn", tb)] = xn
            if tb == 0:
                # Wh right after the first x tile (first 32 matmuls per
                # block touch only Wh), Wl a block later, bias after that
                nc.sync.dma_start(wh[:], wht)
            elif tb == 1:
                nc.sync.dma_start(wl[:], wlt)
            elif tb == 2:
                nc.sync.dma_start(bias_bc[:],
                                  b.unsqueeze(0).partition_broadcast(128))

        def emit_split(tb):
            # token-major hi/lo split for xbar-path blocks
            if tb in PE_BLOCKS:
                return
            xn = state[("xn", tb)]
            hi = hl_pool.tile([128, HID], BF16, name=f"hi_{tb}", tag="hi")
            lo = hl_pool.tile([128, HID], BF16, name=f"lo_{tb}", tag="lo")
            nc.scalar.copy(hi[:], xn[:])
            nc.gpsimd.tensor_tensor(out=lo[:], in0=xn[:], in1=hi[:],
                                    op=ALU.subtract)
            state[("hi", tb)] = hi
            state[("lo", tb)] = lo

        def emit_transpose(tb):
            xh = xt_pool.tile([128, NK, 128], BF16, name=f"xh_{tb}", tag="xh")
            xl = xt_pool.tile([128, NK, 128], BF16, name=f"xl_{tb}", tag="xl")
            if tb in PE_BLOCKS:
                # PE path: fp32 transpose of x, split during evacuation
                xn = state[("xn", tb)]
                for kg in range(NK // 4):
                    px = ps_tp.tile([128, 4, 128], F32, name=f"px_{tb}_{kg}",
                                    tag="ps_tp")
                    for j in range(4):
                        k = kg * 4 + j
                        nc.tensor.transpose(px[:, j, :],
                                            xn[:, k * 128:(k + 1) * 128],
                                            ident[:])
                    sl = slice(kg * 4, kg * 4 + 4)
                    nc.scalar.copy(xh[:, sl, :], px[:])
                    nc.vector.tensor_tensor(out=xl[:, sl, :], in0=px[:],
                                            in1=xh[:, sl, :],
                                            op=ALU.subtract)
            else:
                nc.sync.dma_start(xh[:], state.pop(("hi", tb))[:],
                                  transpose=True)
                nc.sync.dma_start(xl[:], state.pop(("lo", tb))[:],
                                  transpose=True)
            state[("xh", tb)] = xh
            state[("xl", tb)] = xl

        def emit_mm(tb):
            xh = state.pop(("xh", tb))
            xl = state.pop(("xl", tb))
            ps = ps_mm.tile([128, NE], F32, name=f"ps_{tb}", tag="ps_mm")
            # xh passes first: for xbar blocks the xl tiles (second xbar
            # DMA) arrive while the first 32 matmuls already stream
            n = 3 * NK
            i = 0
            for xs, ws in ((xh, wh), (xh, wl), (xl, wh)):
                for k in range(NK):
                    nc.tensor.matmul(ps[:], xs[:, k, :], ws[:, k, :],
                                     start=(i == 0), stop=(i == n - 1))
                    i += 1
            st = st_pool.tile([128, NE], F32, name=f"st_{tb}", tag="st")
            nc.scalar.activation(st[:], ps[:], SIG)
            state[("st", tb)] = st

        def emit_route(tb):
            st = state.pop(("st", tb))
            ssel = rt_pool.tile([128, NE], F32, name=f"ssel_{tb}", tag="ssel")
            nc.vector.tensor_tensor(out=ssel[:], in0=st[:], in1=bias_bc[:],
                                    op=ALU.add)
            gtop = rt_pool.tile([128, NG, 8], F32, name=f"gtop_{tb}",
                                tag="gtop")
            for g in range(NG):
                nc.vector.max(gtop[:, g, :], ssel[:, g * EPG:(g + 1) * EPG])
            g2 = rt_pool.tile([128, NG], F32, name=f"g2_{tb}", tag="g2")
            nc.vector.tensor_tensor(out=g2[:], in0=gtop[:, :, 0],
                                    in1=gtop[:, :, 1], op=ALU.add)
            gs8 = rt_pool.tile([128, NG], F32, name=f"gs8_{tb}", tag="gs8")
            nc.vector.max(gs8[:], g2[:])
            # additive group mask: selected -> 0, unselected -> -BIG
            maskg = rt_pool.tile([128, NG], F32, name=f"mg_{tb}", tag="mg")
            nc.vector.tensor_scalar(out=maskg[:], in0=g2[:],
                                    scalar1=gs8[:, 3:4], scalar2=BIG,
                                    op0=ALU.is_ge, op1=ALU.mult)
            masked = rt_pool.tile([128, NE], F32, name=f"msk_{tb}", tag="msk")
            nc.vector.scalar_tensor_tensor(
                out=masked[:].rearrange("p (g e) -> p g e", g=NG),
                in0=maskg[:].unsqueeze(2).broadcast_to((128, NG, EPG)),
                scalar=BIG,
                in1=ssel[:].rearrange("p (g e) -> p g e", g=NG),
                op0=ALU.subtract, op1=ALU.add)
            top8v = rt_pool.tile([128, TOPK], F32, name=f"t8_{tb}", tag="t8")
            nc.vector.max(top8v[:], masked[:])
            nc.vector.max_index(io[:, tb, :], top8v[:], masked[:])
            ssum = rt_pool.tile([128, 1], F32, name=f"ssum_{tb}", tag="ssum")
            nc.vector.reduce_sum(out=ssum[:], in_=top8v[:],
                                 axis=mybir.AxisListType.X)
            seps = rt_pool.tile([128, 1], F32, name=f"seps_{tb}", tag="seps")
            nc.vector.tensor_scalar_add(seps[:], ssum[:], 1e-6)
            rinv = rt_pool.tile([128, 1], F32, name=f"rinv_{tb}", tag="rinv")
            nc.vector.reciprocal(rinv[:], seps[:])
            nc.vector.tensor_scalar(out=wo[:, tb, :], in0=top8v[:],
                                    scalar1=rinv[:], scalar2=ROUTE_SCALE,
                                    op0=ALU.mult, op1=ALU.mult)

        # ---- software pipeline over token blocks ----
        # xbar transposes go on the SP queue BEFORE the next x load: they
        # gate the PE while the load only feeds two stages later.
        ow = out_w.rearrange("(tb p) k -> p tb k", tb=NB)
        oi = out_i.rearrange("(tb p) k -> p tb k", tb=NB)

        def emit_flush(lo_b, hi_b):
            nc.sync.dma_start(ow[:, lo_b:hi_b, :], wo[:, lo_b:hi_b, :])
            nc.sync.dma_start(oi[:, lo_b:hi_b, :],
                              io[:, lo_b:hi_b, :].bitcast(I32))

        for i in range(NB + 5):
            if 0 <= i - 2 < NB:
                emit_transpose(i - 2)
            if i < NB:
                emit_load(i)
            if 0 <= i - 1 < NB:
                emit_split(i - 1)
            if 0 <= i - 4 < NB:
                emit_mm(i - 4)
            if 0 <= i - 5 < NB:
                emit_route(i - 5)
            if i - 5 == 10:
                emit_flush(0, 11)
        emit_flush(11, NB)


_NC_CACHE = None


def _get_nc():
    global _NC_CACHE
    if _NC_CACHE is None:
        _NC_CACHE = build_nc()
    return _NC_CACHE


def _split_weight(weight):
    """Host weight prep: W [E, H] fp32 -> transposed bf16 hi/lo pair,
    laid out [128, NK, NE] with [p, k, e] = W[e, k*128 + p]."""
    import ml_dtypes
    wt = np.ascontiguousarray(weight.T.astype(np.float32))       # [H, E]
    wh = wt.astype(ml_dtypes.bfloat16)
    wl = (wt - wh.astype(np.float32)).astype(ml_dtypes.bfloat16)
    wh = np.ascontiguousarray(wh.reshape(NK, 128, NE).transpose(1, 0, 2))
    wl = np.ascontiguousarray(wl.reshape(NK, 128, NE).transpose(1, 0, 2))
    return wh, wl


def kernel(hidden_states: np.ndarray, weight: np.ndarray, bias: np.ndarray):
    hidden_states = np.ascontiguousarray(hidden_states, dtype=np.float32)
    weight = np.ascontiguousarray(weight, dtype=np.float32)
    bias = np.ascontiguousarray(bias, dtype=np.float32)
    wh, wl = _split_weight(weight)
    nc = _get_nc()
    in_maps = [
        {
            "hidden_states": hidden_states[c * T_CORE:(c + 1) * T_CORE],
            "wht": wh,
            "wlt": wl,
            "bias": bias,
        }
        for c in range(N_CORES)
    ]
    res = run_bass_kernel_spmd(nc, in_maps, list(range(N_CORES))).results
    weights = np.concatenate([r["weights_out"] for r in res], axis=0)
    indices = np.concatenate([r["indices_out"] for r in res], axis=0)
    return weights.astype(np.float32), indices.astype(np.int32)


# revision 9
# speedup vs baseline: 1.0742x; 1.0002x over previous
"""MiniDeepSeekV3Gate (noaux-topk MoE routing) Trainium2 Bass kernel.

Problem: T=16384 tokens, H=2048 hidden, E=256 experts, 8 groups of 32,
top-2-per-group sums -> top-4 groups -> top-8 experts -> normalized
sigmoid gate weights (scaled 2.5) + int32 expert indices.

Sharding: pure data parallel over tokens. Each of the 8 NeuronCores gets
2048 tokens and a replicated copy of the gate weight (host-prepped as a
transposed bf16 hi/lo pair) + bias. No cross-core communication.

Per-core dataflow (3-pass bf16 split matmul):
  - x is split into bf16 hi = rn(x), lo = rn(x - hi); W likewise into
    Wh + Wl on the host. scores = Wh.xh + Wl.xh + Wh.xl in fp32 PSUM
    (dropped Wl.xl term is ~2^-18 relative: far below the top-k
    tie-break sensitivity that rules out tf32/bf16 single-pass).
    bf16 matmuls stream at 1 cyc/row vs 4 for fp32, so 3 passes cost
    75% of one fp32 pass.
  - matmul orientation: x^T chunks are the STATIONARY operand, W^T
    [128h, 256e] the moving one -> PSUM scores come out token-major
    [128t, 256e], so no transpose is needed between scores and routing.
  - the hi/lo transposes to hidden-major are split between the DMA xbar
    transpose unit (16-bit SBUF->SBUF, 14ns per 16x128 tile) and the PE
    (fp32 transpose of x then split during PSUM evacuation): DMA copies
    serialize on a single 360GB/s resource that also carries the 16MB
    x load, while PE carries the 196k-cycle matmul stream -> balance.
  - routing chain per 128-token block on VectorE: per-group Max8 ->
    group top-2 sums -> top-4 group threshold mask -> masked
    Max8/MaxIndex over 256 -> normalize.  lo-subtracts for xbar blocks
    run on GpSimd to keep VectorE headroom.
"""

import numpy as np

import concourse.bass as bass
import concourse.tile as tile
from concourse import bacc, mybir
from concourse.bass_utils import run_bass_kernel_spmd
from concourse.masks import make_identity

F32 = mybir.dt.float32
BF16 = mybir.dt.bfloat16
I32 = mybir.dt.int32
U32 = mybir.dt.uint32
SIG = mybir.ActivationFunctionType.Sigmoid
ALU = mybir.AluOpType

N_CORES = 8
T_FULL = 16384
T_CORE = T_FULL // N_CORES  # 2048
HID = 2048
NE = 256
NG = 8
EPG = 32
TOPK = 8
ROUTE_SCALE = 2.5
NK = HID // 128          # 16 contraction chunks
NB = T_CORE // 128       # 16 token blocks per core
BIG = 1.0e30

# token blocks whose hi/lo transposes run on the PE (fp32 transpose +
# split-during-evacuation); the rest go through the DMA xbar transpose.
PE_BLOCKS = frozenset((0, 3, 6, 9, 12))


def build_nc():
    nc = bacc.Bacc("TRN2", target_bir_lowering=False, debug=False,
                   num_devices=N_CORES)
    x = nc.dram_tensor("hidden_states", [T_CORE, HID], F32,
                       kind="ExternalInput").ap()
    wht = nc.dram_tensor("wht", [128, NK, NE], BF16, kind="ExternalInput").ap()
    wlt = nc.dram_tensor("wlt", [128, NK, NE], BF16, kind="ExternalInput").ap()
    b = nc.dram_tensor("bias", [NE], F32, kind="ExternalInput").ap()
    out_w = nc.dram_tensor("weights_out", [T_CORE, TOPK], F32,
                           kind="ExternalOutput").ap()
    out_i = nc.dram_tensor("indices_out", [T_CORE, TOPK], I32,
                           kind="ExternalOutput").ap()

    with tile.TileContext(nc) as tc:
        build_tile_kernel(tc, x, wht, wlt, b, out_w, out_i)
    nc.compile()
    return nc


def build_tile_kernel(tc, x, wht, wlt, b, out_w, out_i):
    nc = tc.nc
    from contextlib import ExitStack
    ctx = ExitStack()
    with ctx:
        consts = ctx.enter_context(tc.tile_pool(name="consts", bufs=1))
        xn_pool = ctx.enter_context(tc.tile_pool(name="xn", bufs=4))
        hl_pool = ctx.enter_context(tc.tile_pool(name="hl", bufs=4))
        xt_pool = ctx.enter_context(tc.tile_pool(name="xt", bufs=5))
        st_pool = ctx.enter_context(tc.tile_pool(name="st", bufs=3))
        rt_pool = ctx.enter_context(tc.tile_pool(name="rt", bufs=3))
        ps_mm = ctx.enter_context(tc.tile_pool(name="ps_mm", bufs=3,
                                               space="PSUM"))
        ps_tp = ctx.enter_context(tc.tile_pool(name="ps_tp", bufs=4,
                                               space="PSUM"))

        # ---- constants / weights ----
        ident = consts.tile([128, 128], F32)
        make_identity(nc, ident[:])
        wh = consts.tile([128, NK, NE], BF16)
        wl = consts.tile([128, NK, NE], BF16)
        bias_bc = consts.tile([128, NE], F32)
        # staging for the outputs (written per block, flushed once)
        wo = consts.tile([128, NB, TOPK], F32)
        io = consts.tile([128, NB, TOPK], U32)

        state = {}

        def emit_load(tb):
            xn = xn_pool.tile([128, HID], F32, name=f"xn_{tb}", tag="xn")
            if tb == 0:
                # chunked first load: the PE can start transposing block 0
                # ~2us earlier than a monolithic 1MB DMA allows
                for q in range(4):
                    nc.sync.dma_start(xn[:, q * 512:(q + 1) * 512],
                                      x[0:128, q * 512:(q + 1) * 512])
            else:
                nc.sync.dma_start(xn[:], x[tb * 128:(tb + 1) * 128, :])
            state[("xn", tb)] = xn
            if tb == 0:
                # Wh right after the first x tile (first 32 matmuls per
                # block touch only Wh), Wl a block later, bias after that
                nc.sync.dma_start(wh[:], wht)
            elif tb == 1:
                nc.sync.dma_start(wl[:], wlt)
            elif tb == 2:
                nc.sync.dma_start(bias_bc[:],
                                  b.unsqueeze(0).partition_broadcast(128))

        def emit_split(tb):
            # token-major hi/lo split for xbar-path blocks
            if tb in PE_BLOCKS:
                return
            xn = state[("xn", tb)]
            hi = hl_pool.tile([128, HID], BF16, name=f"hi_{tb}", tag="hi")
            lo = hl_pool.tile([128, HID], BF16, name=f"lo_{tb}", tag="lo")
            nc.scalar.copy(hi[:], xn[:])
            nc.gpsimd.tensor_tensor(out=lo[:], in0=xn[:], in1=hi[:],
                                    op=ALU.subtract)
            state[("hi", tb)] = hi
            state[("lo", tb)] = lo

        def emit_transpose_dma(tb):
            if tb in PE_BLOCKS:
                return
            xh = xt_pool.tile([128, NK, 128], BF16, name=f"xh_{tb}", tag="xh")
            xl = xt_pool.tile([128, NK, 128], BF16, name=f"xl_{tb}", tag="xl")
            nc.sync.dma_start(xh[:], state.pop(("hi", tb))[:], transpose=True)
            nc.sync.dma_start(xl[:], state.pop(("lo", tb))[:], transpose=True)
            state[("xh", tb)] = xh
            state[("xl", tb)] = xl

        def emit_transpose_pe(tb):
            if tb not in PE_BLOCKS:
                return
            xh = xt_pool.tile([128, NK, 128], BF16, name=f"xh_{tb}", tag="xh")
            xl = xt_pool.tile([128, NK, 128], BF16, name=f"xl_{tb}", tag="xl")
            # fp32 transpose of x, hi/lo split during PSUM evacuation
            xn = state[("xn", tb)]
            for kg in range(NK // 4):
                px = ps_tp.tile([128, 4, 128], F32, name=f"px_{tb}_{kg}",
                                tag="ps_tp")
                for j in range(4):
                    k = kg * 4 + j
                    nc.tensor.transpose(px[:, j, :],
                                        xn[:, k * 128:(k + 1) * 128],
                                        ident[:])
                sl = slice(kg * 4, kg * 4 + 4)
                nc.scalar.copy(xh[:, sl, :], px[:])
                nc.vector.tensor_tensor(out=xl[:, sl, :], in0=px[:],
                                        in1=xh[:, sl, :],
                                        op=ALU.subtract)
            state[("xh", tb)] = xh
            state[("xl", tb)] = xl

        def emit_mm(tb):
            xh = state.pop(("xh", tb))
            xl = state.pop(("xl", tb))
            ps = ps_mm.tile([128, NE], F32, name=f"ps_{tb}", tag="ps_mm")
            # xh passes first: for xbar blocks the xl tiles (second xbar
            # DMA) arrive while the first 32 matmuls already stream
            n = 3 * NK
            i = 0
            for xs, ws in ((xh, wh), (xh, wl), (xl, wh)):
                for k in range(NK):
                    nc.tensor.matmul(ps[:], xs[:, k, :], ws[:, k, :],
                                     start=(i == 0), stop=(i == n - 1))
                    i += 1
            st = st_pool.tile([128, NE], F32, name=f"st_{tb}", tag="st")
            nc.scalar.activation(st[:], ps[:], SIG)
            state[("st", tb)] = st

        def emit_route(tb):
            st = state.pop(("st", tb))
            ssel = rt_pool.tile([128, NE], F32, name=f"ssel_{tb}", tag="ssel")
            nc.vector.tensor_tensor(out=ssel[:], in0=st[:], in1=bias_bc[:],
                                    op=ALU.add)
            gtop = rt_pool.tile([128, NG, 8], F32, name=f"gtop_{tb}",
                                tag="gtop")
            for g in range(NG):
                nc.vector.max(gtop[:, g, :], ssel[:, g * EPG:(g + 1) * EPG])
            g2 = rt_pool.tile([128, NG], F32, name=f"g2_{tb}", tag="g2")
            nc.vector.tensor_tensor(out=g2[:], in0=gtop[:, :, 0],
                                    in1=gtop[:, :, 1], op=ALU.add)
            gs8 = rt_pool.tile([128, NG], F32, name=f"gs8_{tb}", tag="gs8")
            nc.vector.max(gs8[:], g2[:])
            # additive group mask: selected -> 0, unselected -> -BIG
            maskg = rt_pool.tile([128, NG], F32, name=f"mg_{tb}", tag="mg")
            nc.vector.tensor_scalar(out=maskg[:], in0=g2[:],
                                    scalar1=gs8[:, 3:4], scalar2=BIG,
                                    op0=ALU.is_ge, op1=ALU.mult)
            masked = rt_pool.tile([128, NE], F32, name=f"msk_{tb}", tag="msk")
            nc.vector.scalar_tensor_tensor(
                out=masked[:].rearrange("p (g e) -> p g e", g=NG),
                in0=maskg[:].unsqueeze(2).broadcast_to((128, NG, EPG)),
                scalar=BIG,
                in1=ssel[:].rearrange("p (g e) -> p g e", g=NG),
                op0=ALU.subtract, op1=ALU.add)
            top8v = rt_pool.tile([128, TOPK], F32, name=f"t8_{tb}", tag="t8")
            nc.vector.max(top8v[:], masked[:])
            nc.vector.max_index(io[:, tb, :], top8v[:], masked[:])
            ssum = rt_pool.tile([128, 1], F32, name=f"ssum_{tb}", tag="ssum")
            nc.vector.reduce_sum(out=ssum[:], in_=top8v[:],
                                 axis=mybir.AxisListType.X)
            seps = rt_pool.tile([128, 1], F32, name=f"seps_{tb}", tag="seps")
            nc.vector.tensor_scalar_add(seps[:], ssum[:], 1e-6)
            rinv = rt_pool.tile([128, 1], F32, name=f"rinv_{tb}", tag="rinv")
            nc.vector.reciprocal(rinv[:], seps[:])
            nc.vector.tensor_scalar(out=wo[:, tb, :], in0=top8v[:],
                                    scalar1=rinv[:], scalar2=ROUTE_SCALE,
                                    op0=ALU.mult, op1=ALU.mult)

        # ---- software pipeline over token blocks ----
        # xbar transposes go on the SP queue BEFORE the next x load: they
        # gate the PE while the load only feeds two stages later.
        ow = out_w.rearrange("(tb p) k -> p tb k", tb=NB)
        oi = out_i.rearrange("(tb p) k -> p tb k", tb=NB)

        def emit_flush(lo_b, hi_b):
            nc.sync.dma_start(ow[:, lo_b:hi_b, :], wo[:, lo_b:hi_b, :])
            nc.sync.dma_start(oi[:, lo_b:hi_b, :],
                              io[:, lo_b:hi_b, :].bitcast(I32))

        # within an iteration: the hi/lo split first (its x tile landed an
        # iteration ago), then the xbar DMAs for a block whose split
        # finished ~2 iterations ago (so the in-order SP queue never parks
        # on an unmet semaphore in front of independent loads), then the
        # next load, PE transposes, matmuls, routing.
        for i in range(NB + 5):
            if 0 <= i - 1 < NB:
                emit_split(i - 1)
            if 0 <= i - 2 < NB:
                emit_transpose_dma(i - 2)
            if i < NB:
                emit_load(i)
            if 0 <= i - 2 < NB:
                emit_transpose_pe(i - 2)
            if 0 <= i - 4 < NB:
                emit_mm(i - 4)
            if 0 <= i - 5 < NB:
                emit_route(i - 5)
            if i - 5 == 10:
                emit_flush(0, 11)
            if i - 5 == 14:
                emit_flush(11, 15)
        emit_flush(15, NB)


_NC_CACHE = None


def _get_nc():
    global _NC_CACHE
    if _NC_CACHE is None:
        _NC_CACHE = build_nc()
    return _NC_CACHE


def _split_weight(weight):
    """Host weight prep: W [E, H] fp32 -> transposed bf16 hi/lo pair,
    laid out [128, NK, NE] with [p, k, e] = W[e, k*128 + p]."""
    import ml_dtypes
    wt = np.ascontiguousarray(weight.T.astype(np.float32))       # [H, E]
    wh = wt.astype(ml_dtypes.bfloat16)
    wl = (wt - wh.astype(np.float32)).astype(ml_dtypes.bfloat16)
    wh = np.ascontiguousarray(wh.reshape(NK, 128, NE).transpose(1, 0, 2))
    wl = np.ascontiguousarray(wl.reshape(NK, 128, NE).transpose(1, 0, 2))
    return wh, wl


def kernel(hidden_states: np.ndarray, weight: np.ndarray, bias: np.ndarray):
    hidden_states = np.ascontiguousarray(hidden_states, dtype=np.float32)
    weight = np.ascontiguousarray(weight, dtype=np.float32)
    bias = np.ascontiguousarray(bias, dtype=np.float32)
    wh, wl = _split_weight(weight)
    nc = _get_nc()
    in_maps = [
        {
            "hidden_states": hidden_states[c * T_CORE:(c + 1) * T_CORE],
            "wht": wh,
            "wlt": wl,
            "bias": bias,
        }
        for c in range(N_CORES)
    ]
    res = run_bass_kernel_spmd(nc, in_maps, list(range(N_CORES))).results
    weights = np.concatenate([r["weights_out"] for r in res], axis=0)
    indices = np.concatenate([r["indices_out"] for r in res], axis=0)
    return weights.astype(np.float32), indices.astype(np.int32)


# revision 10
# speedup vs baseline: 1.0757x; 1.0014x over previous
"""MiniDeepSeekV3Gate (noaux-topk MoE routing) Trainium2 Bass kernel.

Problem: T=16384 tokens, H=2048 hidden, E=256 experts, 8 groups of 32,
top-2-per-group sums -> top-4 groups -> top-8 experts -> normalized
sigmoid gate weights (scaled 2.5) + int32 expert indices.

Sharding: pure data parallel over tokens. Each of the 8 NeuronCores gets
2048 tokens and a replicated copy of the gate weight (host-prepped as a
transposed bf16 hi/lo pair) + bias. No cross-core communication.

Per-core dataflow (3-pass bf16 split matmul):
  - x is split into bf16 hi = rn(x), lo = rn(x - hi); W likewise into
    Wh + Wl on the host. scores = Wh.xh + Wl.xh + Wh.xl in fp32 PSUM
    (dropped Wl.xl term is ~2^-18 relative: far below the top-k
    tie-break sensitivity that rules out tf32/bf16 single-pass).
    bf16 matmuls stream at 1 cyc/row vs 4 for fp32, so 3 passes cost
    75% of one fp32 pass.
  - matmul orientation: x^T chunks are the STATIONARY operand, W^T
    [128h, 256e] the moving one -> PSUM scores come out token-major
    [128t, 256e], so no transpose is needed between scores and routing.
  - the hi/lo transposes to hidden-major are split between the DMA xbar
    transpose unit (16-bit SBUF->SBUF, 14ns per 16x128 tile) and the PE
    (fp32 transpose of x then split during PSUM evacuation): DMA copies
    serialize on a single 360GB/s resource that also carries the 16MB
    x load, while PE carries the 196k-cycle matmul stream -> balance.
  - routing chain per 128-token block on VectorE: per-group Max8 ->
    group top-2 sums -> top-4 group threshold mask -> masked
    Max8/MaxIndex over 256 -> normalize.  lo-subtracts for xbar blocks
    run on GpSimd to keep VectorE headroom.
"""

import numpy as np

import concourse.bass as bass
import concourse.tile as tile
from concourse import bacc, mybir
from concourse.bass_utils import run_bass_kernel_spmd
from concourse.masks import make_identity

F32 = mybir.dt.float32
BF16 = mybir.dt.bfloat16
I32 = mybir.dt.int32
U32 = mybir.dt.uint32
SIG = mybir.ActivationFunctionType.Sigmoid
ALU = mybir.AluOpType

N_CORES = 8
T_FULL = 16384
T_CORE = T_FULL // N_CORES  # 2048
HID = 2048
NE = 256
NG = 8
EPG = 32
TOPK = 8
ROUTE_SCALE = 2.5
NK = HID // 128          # 16 contraction chunks
NB = T_CORE // 128       # 16 token blocks per core
BIG = 1.0e30

# token blocks whose hi/lo transposes run on the PE (fp32 transpose +
# split-during-evacuation); the rest go through the DMA xbar transpose.
PE_BLOCKS = frozenset((0, 3, 6, 9, 12))


def build_nc():
    nc = bacc.Bacc("TRN2", target_bir_lowering=False, debug=False,
                   num_devices=N_CORES)
    x = nc.dram_tensor("hidden_states", [T_CORE, HID], F32,
                       kind="ExternalInput").ap()
    wht = nc.dram_tensor("wht", [128, NK, NE], BF16, kind="ExternalInput").ap()
    wlt = nc.dram_tensor("wlt", [128, NK, NE], BF16, kind="ExternalInput").ap()
    b = nc.dram_tensor("bias", [NE], F32, kind="ExternalInput").ap()
    out_w = nc.dram_tensor("weights_out", [T_CORE, TOPK], F32,
                           kind="ExternalOutput").ap()
    out_i = nc.dram_tensor("indices_out", [T_CORE, TOPK], I32,
                           kind="ExternalOutput").ap()

    with tile.TileContext(nc) as tc:
        build_tile_kernel(tc, x, wht, wlt, b, out_w, out_i)
    nc.compile()
    return nc


def build_tile_kernel(tc, x, wht, wlt, b, out_w, out_i):
    nc = tc.nc
    from contextlib import ExitStack
    ctx = ExitStack()
    with ctx:
        consts = ctx.enter_context(tc.tile_pool(name="consts", bufs=1))
        xn_pool = ctx.enter_context(tc.tile_pool(name="xn", bufs=6))
        hl_pool = ctx.enter_context(tc.tile_pool(name="hl", bufs=6))
        xt_pool = ctx.enter_context(tc.tile_pool(name="xt", bufs=7))
        st_pool = ctx.enter_context(tc.tile_pool(name="st", bufs=3))
        rt_pool = ctx.enter_context(tc.tile_pool(name="rt", bufs=3))
        ps_mm = ctx.enter_context(tc.tile_pool(name="ps_mm", bufs=3,
                                               space="PSUM"))
        ps_tp = ctx.enter_context(tc.tile_pool(name="ps_tp", bufs=4,
                                               space="PSUM"))

        # ---- constants / weights ----
        ident = consts.tile([128, 128], F32)
        make_identity(nc, ident[:])
        wh = consts.tile([128, NK, NE], BF16)
        wl = consts.tile([128, NK, NE], BF16)
        bias_bc = consts.tile([128, NE], F32)
        # staging for the outputs (written per block, flushed once)
        wo = consts.tile([128, NB, TOPK], F32)
        io = consts.tile([128, NB, TOPK], U32)

        state = {}

        def emit_load(tb):
            xn = xn_pool.tile([128, HID], F32, name=f"xn_{tb}", tag="xn")
            if tb == 0:
                # chunked first load: the PE can start transposing block 0
                # ~2us earlier than a monolithic 1MB DMA allows
                for q in range(4):
                    nc.sync.dma_start(xn[:, q * 512:(q + 1) * 512],
                                      x[0:128, q * 512:(q + 1) * 512])
            else:
                nc.sync.dma_start(xn[:], x[tb * 128:(tb + 1) * 128, :])
            state[("xn", tb)] = xn
            if tb == 0:
                # Wh right after the first x tile (first 32 matmuls per
                # block touch only Wh), Wl a block later, bias after that
                nc.sync.dma_start(wh[:], wht)
            elif tb == 1:
                nc.sync.dma_start(wl[:], wlt)
            elif tb == 2:
                nc.sync.dma_start(bias_bc[:],
                                  b.unsqueeze(0).partition_broadcast(128))

        def emit_split(tb):
            # token-major hi/lo split for xbar-path blocks
            if tb in PE_BLOCKS:
                return
            xn = state[("xn", tb)]
            hi = hl_pool.tile([128, HID], BF16, name=f"hi_{tb}", tag="hi")
            lo = hl_pool.tile([128, HID], BF16, name=f"lo_{tb}", tag="lo")
            nc.scalar.copy(hi[:], xn[:])
            nc.gpsimd.tensor_tensor(out=lo[:], in0=xn[:], in1=hi[:],
                                    op=ALU.subtract)
            state[("hi", tb)] = hi
            state[("lo", tb)] = lo

        def emit_transpose_dma(tb):
            if tb in PE_BLOCKS:
                return
            xh = xt_pool.tile([128, NK, 128], BF16, name=f"xh_{tb}", tag="xh")
            xl = xt_pool.tile([128, NK, 128], BF16, name=f"xl_{tb}", tag="xl")
            nc.sync.dma_start(xh[:], state.pop(("hi", tb))[:], transpose=True)
            nc.sync.dma_start(xl[:], state.pop(("lo", tb))[:], transpose=True)
            state[("xh", tb)] = xh
            state[("xl", tb)] = xl

        def emit_transpose_pe(tb):
            if tb not in PE_BLOCKS:
                return
            xh = xt_pool.tile([128, NK, 128], BF16, name=f"xh_{tb}", tag="xh")
            xl = xt_pool.tile([128, NK, 128], BF16, name=f"xl_{tb}", tag="xl")
            # fp32 transpose of x, hi/lo split during PSUM evacuation
            xn = state[("xn", tb)]
            for kg in range(NK // 4):
                px = ps_tp.tile([128, 4, 128], F32, name=f"px_{tb}_{kg}",
                                tag="ps_tp")
                for j in range(4):
                    k = kg * 4 + j
                    nc.tensor.transpose(px[:, j, :],
                                        xn[:, k * 128:(k + 1) * 128],
                                        ident[:])
                sl = slice(kg * 4, kg * 4 + 4)
                nc.scalar.copy(xh[:, sl, :], px[:])
                nc.vector.tensor_tensor(out=xl[:, sl, :], in0=px[:],
                                        in1=xh[:, sl, :],
                                        op=ALU.subtract)
            state[("xh", tb)] = xh
            state[("xl", tb)] = xl

        def emit_mm(tb):
            xh = state.pop(("xh", tb))
            xl = state.pop(("xl", tb))
            ps = ps_mm.tile([128, NE], F32, name=f"ps_{tb}", tag="ps_mm")
            # xh passes first: for xbar blocks the xl tiles (second xbar
            # DMA) arrive while the first 32 matmuls already stream
            n = 3 * NK
            i = 0
            for xs, ws in ((xh, wh), (xh, wl), (xl, wh)):
                for k in range(NK):
                    nc.tensor.matmul(ps[:], xs[:, k, :], ws[:, k, :],
                                     start=(i == 0), stop=(i == n - 1))
                    i += 1
            st = st_pool.tile([128, NE], F32, name=f"st_{tb}", tag="st")
            nc.scalar.activation(st[:], ps[:], SIG)
            state[("st", tb)] = st

        def emit_route(tb):
            st = state.pop(("st", tb))
            ssel = rt_pool.tile([128, NE], F32, name=f"ssel_{tb}", tag="ssel")
            nc.vector.tensor_tensor(out=ssel[:], in0=st[:], in1=bias_bc[:],
                                    op=ALU.add)
            gtop = rt_pool.tile([128, NG, 8], F32, name=f"gtop_{tb}",
                                tag="gtop")
            for g in range(NG):
                nc.vector.max(gtop[:, g, :], ssel[:, g * EPG:(g + 1) * EPG])
            g2 = rt_pool.tile([128, NG], F32, name=f"g2_{tb}", tag="g2")
            nc.vector.tensor_tensor(out=g2[:], in0=gtop[:, :, 0],
                                    in1=gtop[:, :, 1], op=ALU.add)
            gs8 = rt_pool.tile([128, NG], F32, name=f"gs8_{tb}", tag="gs8")
            nc.vector.max(gs8[:], g2[:])
            # additive group mask: selected -> 0, unselected -> -BIG
            maskg = rt_pool.tile([128, NG], F32, name=f"mg_{tb}", tag="mg")
            nc.vector.tensor_scalar(out=maskg[:], in0=g2[:],
                                    scalar1=gs8[:, 3:4], scalar2=BIG,
                                    op0=ALU.is_ge, op1=ALU.mult)
            masked = rt_pool.tile([128, NE], F32, name=f"msk_{tb}", tag="msk")
            nc.vector.scalar_tensor_tensor(
                out=masked[:].rearrange("p (g e) -> p g e", g=NG),
                in0=maskg[:].unsqueeze(2).broadcast_to((128, NG, EPG)),
                scalar=BIG,
                in1=ssel[:].rearrange("p (g e) -> p g e", g=NG),
                op0=ALU.subtract, op1=ALU.add)
            top8v = rt_pool.tile([128, TOPK], F32, name=f"t8_{tb}", tag="t8")
            nc.vector.max(top8v[:], masked[:])
            nc.vector.max_index(io[:, tb, :], top8v[:], masked[:])
            ssum = rt_pool.tile([128, 1], F32, name=f"ssum_{tb}", tag="ssum")
            nc.vector.reduce_sum(out=ssum[:], in_=top8v[:],
                                 axis=mybir.AxisListType.X)
            seps = rt_pool.tile([128, 1], F32, name=f"seps_{tb}", tag="seps")
            nc.vector.tensor_scalar_add(seps[:], ssum[:], 1e-6)
            rinv = rt_pool.tile([128, 1], F32, name=f"rinv_{tb}", tag="rinv")
            nc.vector.reciprocal(rinv[:], seps[:])
            nc.vector.tensor_scalar(out=wo[:, tb, :], in0=top8v[:],
                                    scalar1=rinv[:], scalar2=ROUTE_SCALE,
                                    op0=ALU.mult, op1=ALU.mult)

        # ---- software pipeline over token blocks ----
        # xbar transposes go on the SP queue BEFORE the next x load: they
        # gate the PE while the load only feeds two stages later.
        ow = out_w.rearrange("(tb p) k -> p tb k", tb=NB)
        oi = out_i.rearrange("(tb p) k -> p tb k", tb=NB)

        def emit_flush(lo_b, hi_b):
            nc.sync.dma_start(ow[:, lo_b:hi_b, :], wo[:, lo_b:hi_b, :])
            nc.sync.dma_start(oi[:, lo_b:hi_b, :],
                              io[:, lo_b:hi_b, :].bitcast(I32))

        # within an iteration: the hi/lo split first (its x tile landed an
        # iteration ago), then the xbar DMAs for a block whose split
        # finished ~2 iterations ago (so the in-order SP queue never parks
        # on an unmet semaphore in front of independent loads), then the
        # next load, PE transposes, matmuls, routing.
        for i in range(NB + 5):
            if 0 <= i - 1 < NB:
                emit_split(i - 1)
            if 0 <= i - 2 < NB:
                emit_transpose_dma(i - 2)
            if i < NB:
                emit_load(i)
            if 0 <= i - 2 < NB:
                emit_transpose_pe(i - 2)
            if 0 <= i - 4 < NB:
                emit_mm(i - 4)
            if 0 <= i - 5 < NB:
                emit_route(i - 5)
            if i - 5 == 10:
                emit_flush(0, 11)
            if i - 5 == 14:
                emit_flush(11, 15)
        emit_flush(15, NB)


_NC_CACHE = None


def _get_nc():
    global _NC_CACHE
    if _NC_CACHE is None:
        _NC_CACHE = build_nc()
    return _NC_CACHE


def _split_weight(weight):
    """Host weight prep: W [E, H] fp32 -> transposed bf16 hi/lo pair,
    laid out [128, NK, NE] with [p, k, e] = W[e, k*128 + p]."""
    import ml_dtypes
    wt = np.ascontiguousarray(weight.T.astype(np.float32))       # [H, E]
    wh = wt.astype(ml_dtypes.bfloat16)
    wl = (wt - wh.astype(np.float32)).astype(ml_dtypes.bfloat16)
    wh = np.ascontiguousarray(wh.reshape(NK, 128, NE).transpose(1, 0, 2))
    wl = np.ascontiguousarray(wl.reshape(NK, 128, NE).transpose(1, 0, 2))
    return wh, wl


def kernel(hidden_states: np.ndarray, weight: np.ndarray, bias: np.ndarray):
    hidden_states = np.ascontiguousarray(hidden_states, dtype=np.float32)
    weight = np.ascontiguousarray(weight, dtype=np.float32)
    bias = np.ascontiguousarray(bias, dtype=np.float32)
    wh, wl = _split_weight(weight)
    nc = _get_nc()
    in_maps = [
        {
            "hidden_states": hidden_states[c * T_CORE:(c + 1) * T_CORE],
            "wht": wh,
            "wlt": wl,
            "bias": bias,
        }
        for c in range(N_CORES)
    ]
    res = run_bass_kernel_spmd(nc, in_maps, list(range(N_CORES))).results
    weights = np.concatenate([r["weights_out"] for r in res], axis=0)
    indices = np.concatenate([r["indices_out"] for r in res], axis=0)
    return weights.astype(np.float32), indices.astype(np.int32)
